# revision 84
# baseline (speedup 1.0000x reference)
"""Trainium2 Bass kernel for nn_ExtremeOptimizationLayer (64-branch MLP + per-branch
BatchNorm + fusion gate), SPMD across 8 NeuronCores.

Sharding: expert-parallel over the 64 branches (8 per core). Per core:
  GEMM1: h_k = relu(x @ W_k + b_k) for the 8 local branches, full batch,
         computing BN batch stats on the fly (bn_stats/bn_aggr).
  BN is folded into the fusion GEMM: h_norm = h*u + v with
         u = gamma*rsqrt(var+eps), v = beta - mean*u, so
         z_partial = h @ (u*Wf1_local) + (v @ Wf1_local).
  The rank-1 term zc = v @ Wf1_local is computed with column-packed M=1
  matmuls (4 concurrent PE column groups), summed across cores by a tiny
  AllReduce, and folded into the post-RS relu bias.
  GEMM2: z_partial[b, j] over the local 8192 rows of Wf1.
  ReduceScatter (bf16 wire, overlapped with compute) sums the 8 partial
  z's; each core ends with 256 batch rows, applies relu(z+bf1+zc) and the
  final GEMM with Wf2.
All matmuls run in bf16 (fp32 PSUM accumulation).

Schedule notes (tuned against HW traces):
 - h staging in DRAM is split into one tensor per 4-ko-tile group so the
   GEMM2 reloads only fence on the branches that produced them; the sync
   queue prefetches GEMM2's first h tiles while GEMM1 still runs, so the
   PE never idles (and never HAM-rethrottles) at the phase boundary.
 - z partials travel as bf16 through the ReduceScatter (half the wire
   and DMA bytes; adds ~0.1-0.3% relative error, well inside tolerance).
 - The final GEMM is split: local batch rows 0..191 only need collectives
   that finish before the last RS, so their transposes + matmuls + output
   DMA run in the latency shadow of the final ReduceScatter; only the
   last 64 rows (the last b-macro's shard) remain on the critical path.
 - zs (post-RS) loads ride the vector DMA queue, which has no other
   phase-B work, so a load that blocks on an in-flight RS never stalls
   the drain/store pipelines.
DMA queue split: weights/consts on gpsimd, xT + h on sync(+vector at
startup), h writes + z stores + out on scalar, zs loads on vector.
"""

import numpy as np
import ml_dtypes

import concourse.bass as bass
import concourse.mybir as mybir
import concourse.tile as tile
from concourse import bacc
from concourse.bass_utils import run_bass_kernel_spmd
from concourse.masks import make_identity

F32 = mybir.dt.float32
BF16 = mybir.dt.bfloat16
BD = ml_dtypes.bfloat16
AF = mybir.ActivationFunctionType

FULL_CFG = dict(
    n_cores=8, B=2048, DI=1024, DO=1024, KT=64, DF=1024, DO2=1024,
    JCH=2, EPS=1e-5,
)


def _dims(cfg):
    d = dict(cfg)
    d["KB"] = cfg["KT"] // cfg["n_cores"]        # branches per core
    d["TPB"] = cfg["DO"] // 128                  # o-tiles per branch
    d["T"] = d["KB"] * d["TPB"]                  # local ko tiles
    d["NIT"] = cfg["DI"] // 128                  # i-tiles (GEMM1 contraction)
    d["BC"] = min(512, cfg["B"])                 # GEMM1 batch chunk
    d["NBC"] = cfg["B"] // d["BC"]
    d["JW"] = cfg["DF"] // cfg["JCH"]            # j-chunk width
    d["BMW"] = min(512, cfg["B"])                # GEMM2 b-macro width
    d["NBM"] = cfg["B"] // d["BMW"]
    d["NBT"] = d["BMW"] // 128                   # b-tiles per macro
    d["PR"] = min(256, d["BMW"])                 # RS piece rows
    d["NPS"] = d["BMW"] // d["PR"]               # RS pieces per b-macro
    d["BL"] = cfg["B"] // cfg["n_cores"]         # local batch rows after RS
    d["PL"] = d["PR"] // cfg["n_cores"]          # local rows per RS piece
    d["NJT"] = cfg["DF"] // 128                  # final j-tiles
    d["NCT"] = cfg["DO2"] // 128                 # final out-col tiles
    # the last chunk's last b-macro splits 1/2 + 1/4 + 1/4 so the final
    # (latency-critical) RS is quarter-size
    d["TSP"] = [d["BMW"] // 2, d["BMW"] // 4, d["BMW"] // 4]
    d["TLW"] = d["TSP"][-1] // cfg["n_cores"]    # post-RS local rows
    d["P1W"] = d["BL"] - d["TLW"]                # pre-RS final-GEMM width
    d["FRW"] = d["BL"] - d["BMW"] // cfg["n_cores"]  # front (full-macro) rows
    d["HQ"] = min(4, d["T"])                     # ko-tiles per h2 macro-DMA
    return d


def build_bass(cfg):
    d = _dims(cfg)
    n_cores, B, DI, DO = cfg["n_cores"], cfg["B"], cfg["DI"], cfg["DO"]
    DF, DO2, JCH, EPS = cfg["DF"], cfg["DO2"], cfg["JCH"], cfg["EPS"]
    KB, TPB, T, NIT = d["KB"], d["TPB"], d["T"], d["NIT"]
    BC, NBC, JW = d["BC"], d["NBC"], d["JW"]
    BMW, NBM, NBT, BL, PL = d["BMW"], d["NBM"], d["NBT"], d["BL"], d["PL"]
    PR, NPS, HQ = d["PR"], d["NPS"], d["HQ"]
    NJT, NCT = d["NJT"], d["NCT"]
    TLW, P1W, FRW = d["TLW"], d["P1W"], d["FRW"]
    NHQ = T // HQ                                 # h macro groups

    nc = bacc.Bacc("TRN2", target_bir_lowering=False, debug=False,
                   num_devices=n_cores)
    # defer the last branch's chunk-0 zc packs into GEMM2 only when the
    # first macro's t-loop is long enough that the u-fold of those tiles
    # (issued a few h-macro-loads in) still precedes their first use
    DEFER = NHQ >= 4

    xT = nc.dram_tensor("xT", [DI, B], BF16, kind="ExternalInput").ap()
    wloc = nc.dram_tensor("wloc", [KB * DI, DO], BF16, kind="ExternalInput").ap()
    w1loc = nc.dram_tensor("w1loc", [KB * DO, DF], BF16, kind="ExternalInput").ap()
    wf2 = nc.dram_tensor("wf2", [DF, DO2], BF16, kind="ExternalInput").ap()
    b_r = nc.dram_tensor("b_r", [128, T], F32, kind="ExternalInput").ap()
    gamma_r = nc.dram_tensor("gamma_r", [128, T], F32, kind="ExternalInput").ap()
    beta_r = nc.dram_tensor("beta_r", [128, T], F32, kind="ExternalInput").ap()
    bf1_r = nc.dram_tensor("bf1_r", [128, NJT], F32, kind="ExternalInput").ap()
    bf2_r = nc.dram_tensor("bf2_r", [128, NCT], F32, kind="ExternalInput").ap()
    bf2_f = nc.dram_tensor("bf2_f", [1, DO2], BF16, kind="ExternalInput").ap()
    # final output, split: rows [0, P1W) land batch-major during the RS
    # shadow; the last TLW rows land channel-major on the critical path
    out_a = nc.dram_tensor("out_a", [P1W, DO2], F32, kind="ExternalOutput").ap()
    out_b = nc.dram_tensor("out_b", [DO2, TLW], F32, kind="ExternalOutput").ap()

    # h staging, one DRAM tensor per HQ-ko-tile macro group so GEMM2's
    # reloads fence only on the branches that wrote them
    hq_d = [nc.dram_tensor(f"hq{q}", [HQ * 128, B], BF16, kind="Internal").ap()
            for q in range(NHQ)]

    # GEMM2 b-macros (col0, width) per j-chunk: all full width, except the
    # LAST chunk's last macro splits (1/2 + 1/4 + 1/4) so the latency-
    # critical final RS is quarter-size (and the rest of its local rows
    # move into the RS shadow)
    TSP = d["TSP"]

    def macs(jc):
        m = [(bm * BMW, BMW) for bm in range(NBM - 1)]
        if jc == JCH - 1:
            c = (NBM - 1) * BMW
            for w in TSP:
                m.append((c, w))
                c += w
        else:
            m += [((NBM - 1) * BMW, BMW)]
        return m

    def pieces(jc, mi):
        # (sub-piece index, row0 within macro, nrows) of the macro's RS
        # parts.  The batch-row partition must be IDENTICAL across j-chunks
        # (each piece's RS lands plp rows per core at a fixed local slot),
        # so the last BMW rows follow the TSP boundaries for every chunk.
        c0, w = macs(jc)[mi]
        lb = (NBM - 1) * BMW
        out, sp, r = [], 0, 0
        while r < w:
            if c0 + r < lb:
                n = min(PR, w - r)
            else:
                off, acc = c0 + r - lb, 0
                for tw in TSP:
                    if off < acc + tw:
                        n = min(acc + tw - off, w - r)
                        break
                    acc += tw
            out.append((sp, r, n))
            sp += 1
            r += n
        return out

    # RS pieces: (j-chunk, macro, sub-piece); bf16 wire
    zp = {}
    zs = {}
    for jc in range(JCH):
        for mi in range(len(macs(jc))):
            for sp, row0, nrows in pieces(jc, mi):
                zp[(jc, mi, sp)] = nc.dram_tensor(
                    f"zp{jc}_{mi}_{sp}", [nrows, JW], BF16, kind="Internal").ap()
                zs[(jc, mi, sp)] = nc.dram_tensor(
                    f"zs{jc}_{mi}_{sp}", [nrows // n_cores, JW], BF16,
                    kind="Internal").ap()
    # 8 packed zc rows: 4 from branches [0, KB-1) folded during GEMM1, 4
    # from the last branch (deferred into GEMM2 so its pack matmuls never
    # stall the PE on the trailing BN-stats chain)
    zc_in = nc.dram_tensor("zc_in", [8, DF], F32, kind="Internal").ap()
    zc_out = nc.dram_tensor("zc_out", [8, DF], F32, kind="Internal").ap()

    with tile.TileContext(nc) as tc:
        with tc.tile_pool(name="const", bufs=1) as cp, \
             tc.tile_pool(name="stats", bufs=1) as sp_pool, \
             tc.tile_pool(name="w1a", bufs=T) as w1a, \
             tc.tile_pool(name="h2", bufs=6) as h2p:
            br_sb = cp.tile([128, T], F32, name="br_sb")
            gr_sb = cp.tile([128, T], F32, name="gr_sb")
            be_sb = cp.tile([128, T], F32, name="be_sb")
            bf1_sb = cp.tile([128, NJT], F32, name="bf1_sb")
            bf2_sb = cp.tile([128, NCT], F32, name="bf2_sb")
            eps_sb = cp.tile([128, 1], F32, name="eps_sb")
            ident = cp.tile([128, 128], BF16, name="ident")
            wf2_sb = cp.tile([128, NJT, DO2], BF16, name="wf2_sb")
            ones_sb = cp.tile([1, 128], BF16, name="ones_sb")
            bf2row = cp.tile([1, DO2], BF16, name="bf2row")

            # stats
            mv = sp_pool.tile([128, T, 2], F32, name="mv")
            u_all = sp_pool.tile([128, T], F32, name="u_all")
            v_f32 = sp_pool.tile([128, T], F32, name="v_f32")
            v_bf = sp_pool.tile([128, T], BF16, name="v_bf")
            zcs_t = [sp_pool.tile([128, JW], F32, name=f"zcs_{jc}")
                     for jc in range(JCH)]
            zcs7 = sp_pool.tile([128, JW], F32, name="zcs7")
            zzero = sp_pool.tile([4, JW], F32, name="zzero")

            w1_tiles = [[None] * T for _ in range(JCH)]

            def zc_pack_mm(zc_ps, jc, t, t_lo, t_hi):
                """zc[j] += v[t-tile] @ Wf1[t-tile, jc-chunk] as an M=1 matmul
                packed into PE column group t%4 (concurrent execution);
                [t_lo, t_hi) is the tile range packed into this PSUM bank."""
                g = t % 4
                nc.tensor.matmul(zc_ps[32 * g:32 * g + 1, :], v_bf[:, t:t + 1],
                                 w1_tiles[jc][t][:],
                                 start=(t < t_lo + 4), stop=(t >= t_hi - 4),
                                 tile_position=(0, 32 * g),
                                 skip_group_check=True)

            def zc_collect(zc_ps, zcs, jc, r0=0):
                """Copy the zc PSUM bank to SBUF (lane-aligned), then DMA the 4
                packed rows (partitions 0/32/64/96) to the AllReduce input."""
                nc.scalar.activation(zcs[:], zc_ps[:], AF.Copy)
                rows = zcs[:].rearrange("(g s) w -> g s w", s=32)[:, 0:1, :]
                nc.scalar.dma_start(zc_in[r0:r0 + 4, jc * JW:(jc + 1) * JW],
                                    rows)

            # ---------------- GEMM1: branch MLPs + BN stats ----------------
            with tc.tile_pool(name="xt", bufs=1) as xtp, \
                 tc.tile_pool(name="w", bufs=2 * NIT) as wp, \
                 tc.tile_pool(name="h1", bufs=8) as hp, \
                 tc.tile_pool(name="bn", bufs=2 * TPB + 2) as bnp, \
                 tc.tile_pool(name="g1ps", bufs=7, space="PSUM") as g1ps, \
                 tc.tile_pool(name="zc0ps", bufs=1, space="PSUM") as zc0ps:
                # branch-0 weights split across scalar+gpsimd for fast start
                w_tiles = []
                nc.scalar.dma_start(br_sb[:], b_r[:, :])
                for it in range(NIT):
                    wt = wp.tile([128, DO], BF16, name=f"w_0_{it}", tag="w")
                    eng = nc.scalar if it % 2 == 0 else nc.gpsimd
                    eng.dma_start(wt[:], wloc[it * 128:(it + 1) * 128, :])
                    w_tiles.append(wt)
                nc.gpsimd.dma_start(gr_sb[:], gamma_r[:, :])
                nc.gpsimd.dma_start(be_sb[:], beta_r[:, :])
                nc.gpsimd.memset(eps_sb[:], EPS)
                # xT bc-major so the first MMs unblock fast; later chunks'
                # odd i-tiles ride scalar so the sync queue can't starve the
                # last batch chunks of branch 0
                xt_sb = xtp.tile([128, NIT, B], BF16, name="xt_sb")
                for bc in range(NBC):
                    for it in range(NIT):
                        eng = nc.scalar if (bc >= 1 and it % 2 == 1) else nc.sync
                        eng.dma_start(
                            xt_sb[:, it, bc * BC:(bc + 1) * BC],
                            xT[it * 128:(it + 1) * 128, bc * BC:(bc + 1) * BC])
                zc0_ps = zc0ps.tile([128, JW], F32, name="zc0_ps")
                nc.vector.memset(zc0_ps[:], 0.0)

                for kb in range(KB):
                    if kb > 0:
                        w_tiles = []
                        for it in range(NIT):
                            wt = wp.tile([128, DO], BF16, name=f"w_{kb}_{it}",
                                         tag="w")
                            nc.gpsimd.dma_start(
                                wt[:],
                                wloc[kb * DI + it * 128:kb * DI + (it + 1) * 128, :])
                            w_tiles.append(wt)
                    # spread Wf1 chunk-0 prefetch across branches (gpsimd)
                    for ot in range(TPB):
                        t = kb * TPB + ot
                        w1t = w1a.tile([128, JW], BF16, name=f"w1_0_{t}",
                                       tag="w1a")
                        nc.gpsimd.dma_start(
                            w1t[:], w1loc[t * 128:(t + 1) * 128, 0:JW])
                        w1_tiles[0][t] = w1t
                    bn6s = [bnp.tile([128, NBC, 6], F32,
                                     name=f"bn6_{kb * TPB + ot}", tag="bn6")
                            for ot in range(TPB)]
                    # branch 0 runs bc-outer so the first matmuls only need the
                    # first xT batch-chunk; later branches run ot-outer
                    if kb == 0:
                        loop = [(ot, bc) for bc in range(NBC) for ot in range(TPB)]
                    else:
                        loop = [(ot, bc) for ot in range(TPB) for bc in range(NBC)]
                    for ot, bc in loop:
                        t = kb * TPB + ot
                        ps = g1ps.tile([128, BC], F32, name=f"g1_{t}_{bc}",
                                       tag="g1")
                        for it in range(NIT):
                            nc.tensor.matmul(
                                ps[:],
                                w_tiles[it][:, ot * 128:(ot + 1) * 128],
                                xt_sb[:, it, bc * BC:(bc + 1) * BC],
                                start=(it == 0), stop=(it == NIT - 1))
                        hsb = hp.tile([128, BC], BF16, name=f"h_{t}_{bc}",
                                      tag="h1")
                        nc.scalar.activation(hsb[:], ps[:], AF.Relu,
                                             bias=br_sb[:, t:t + 1])
                        nc.vector.bn_stats(bn6s[ot][:, bc, :], hsb[:])
                        nc.scalar.dma_start(
                            hq_d[t // HQ][(t % HQ) * 128:(t % HQ) * 128 + 128,
                                          bc * BC:(bc + 1) * BC],
                            hsb[:])
                    for ot in range(TPB):
                        t = kb * TPB + ot
                        nc.vector.bn_aggr(
                            mv[:, t, :],
                            bn6s[ot][:].rearrange("p a (x c) -> p (a x) c", c=3))
                    # per-branch BN affine folding: u = gamma*rsqrt(var+eps),
                    # v = beta - mean*u
                    t0 = kb * TPB
                    stdt = bnp.tile([128, TPB], F32, name=f"std_{kb}", tag="std")
                    nc.scalar.activation(stdt[:], mv[:, t0:t0 + TPB, 1:2],
                                         AF.Sqrt, bias=eps_sb[:])
                    invt = bnp.tile([128, TPB], F32, name=f"inv_{kb}", tag="inv")
                    nc.vector.reciprocal(invt[:], stdt[:])
                    nc.vector.tensor_mul(u_all[:, t0:t0 + TPB], invt[:],
                                         gr_sb[:, t0:t0 + TPB])
                    mut = bnp.tile([128, TPB], F32, name=f"mu_{kb}", tag="mu")
                    nc.vector.tensor_mul(mut[:], mv[:, t0:t0 + TPB, 0:1],
                                         u_all[:, t0:t0 + TPB])
                    nc.vector.tensor_sub(v_f32[:, t0:t0 + TPB],
                                         be_sb[:, t0:t0 + TPB], mut[:])
                    nc.vector.tensor_copy(v_bf[:, t0:t0 + TPB],
                                          v_f32[:, t0:t0 + TPB])
                    # chunk-0 prep (zc, then fold u into Wf1) for the PREVIOUS
                    # branch — its stats chain has finished by now, so the
                    # in-order PE doesn't stall on it.  The LAST branch's
                    # packs are deferred into GEMM2 for the same reason.
                    t_hi0 = T - TPB if DEFER else T
                    for pb in ([kb - 1] if kb > 0 else []) + \
                              ([kb] if (kb == KB - 1 and not DEFER) else []):
                        for ot in range(TPB):
                            t = pb * TPB + ot
                            zc_pack_mm(zc0_ps, 0, t, 0, t_hi0)
                            nc.vector.tensor_scalar_mul(w1_tiles[0][t][:],
                                                        w1_tiles[0][t][:],
                                                        u_all[:, t:t + 1])
                zc_collect(zc0_ps, zcs_t[0], 0)

            # ---------------- GEMM2: fusion gate partials + RS ----------------
            with tc.tile_pool(name="w1b", bufs=(JCH - 1) * T if JCH > 1 else 1) as w1b, \
                 tc.tile_pool(name="zsb", bufs=4) as zsbp, \
                 tc.tile_pool(name="zr", bufs=1) as zrp, \
                 tc.tile_pool(name="fo", bufs=4) as fop, \
                 tc.tile_pool(name="zps", bufs=6, space="PSUM") as zps, \
                 tc.tile_pool(name="tl_ps", bufs=2, space="PSUM") as tlps:
                # consts needed from the middle of GEMM2 on (gpsimd queue)
                nc.gpsimd.dma_start(bf1_sb[:], bf1_r[:, :])
                nc.gpsimd.dma_start(bf2_sb[:], bf2_r[:, :])
                nc.gpsimd.dma_start(bf2row[:], bf2_f[:, :])
                nc.gpsimd.memset(ones_sb[:], 1.0)
                make_identity(nc, ident[:])
                # rows 4:8 of zc_in carry only the last branch's CHUNK-0
                # packs; zero every column range they don't cover
                nc.gpsimd.memset(zzero[:], 0.0)
                for jcx in range(0 if not DEFER else 1, JCH):
                    nc.gpsimd.dma_start(
                        zc_in[4:8, jcx * JW:(jcx + 1) * JW], zzero[:])
                # later chunks' Wf1 DMA (gpsimd queue, ahead of any RS).
                # Only the first half is issued up front: the full 8MB burst
                # contends with the h2 stream for HBM right when GEMM2 is
                # also fighting the first ReduceScatters, and can starve the
                # PE of h tiles.  The rest is issued a macro later, and wf2
                # (tail-only) much later still.
                def w1b_issue(jc, lo, hi):
                    for t in range(lo, hi):
                        w1t = w1b.tile([128, JW], BF16, name=f"w1_{jc}_{t}",
                                       tag="w1b")
                        nc.gpsimd.dma_start(
                            w1t[:],
                            w1loc[t * 128:(t + 1) * 128, jc * JW:(jc + 1) * JW])
                        w1_tiles[jc][t] = w1t
                for jc in range(1, JCH):
                    w1b_issue(jc, 0, T // 2)

                # final-phase state (filled in as RS pieces land): local rows
                # in full-height row tiles (one 128-wide transpose per j-tile
                # each), except the last TLW rows' last-chunk columns, which
                # arrive after the final RS as a jt-packed staging tile.
                zrow = []
                r = 0
                while r < BL:
                    nr = min(128, BL - r)
                    zrow.append((zrp.tile([nr, DF], BF16, name=f"zrow_{r}",
                                          tag=f"zrow{r}"), r, nr))
                    r += nr
                NJL = JW // 128            # j-tiles in the last chunk
                zcb4 = zrp.tile([128, 8, NJT], F32, name="zcb4")
                biasall = zrp.tile([128, NJT], F32, name="biasall")
                zrT = [zrp.tile([128, BL], BF16, name=f"zrT_{jt}",
                                tag=f"zrT{jt}") for jt in range(NJT)]
                # the last tile's post-RS rows are transposed as garbage
                # during the shadow (then overwritten); keep them finite.
                # (32-aligned start partition; the extra rows reload later)
                _mz = (zrow[-1][2] - TLW) // 32 * 32
                nc.vector.memset(zrow[-1][0][_mz:, :], 0.0)

                def zs_load(jc_, mi_, sp_, eng=None):
                    """DMA an RS output shard into its SBUF slot (scalar
                    queue; callers only issue it once the RS has landed)."""
                    _, row0, nrows = next(p for p in pieces(jc_, mi_)
                                          if p[0] == sp_)
                    plp = nrows // n_cores
                    l0 = (macs(jc_)[mi_][0] + row0) // n_cores
                    done = 0
                    while done < plp:  # a shard may straddle row tiles
                        l = l0 + done
                        dstt, base, nr = next(z for z in zrow
                                              if z[1] <= l < z[1] + z[2])
                        take = min(plp - done, base + nr - l)
                        (eng or nc.scalar).dma_start(
                            dstt[l - base:l - base + take,
                                 jc_ * JW:(jc_ + 1) * JW],
                            zs[(jc_, mi_, sp_)][done:done + take, :])
                        done += take

                # MM-slot bounds of every (jc, macro) block, for the flush
                # safety rule: a piece's zs load is issued only >=280 MM
                # slots (~75us) after its RS trigger, so the scalar queue
                # never blocks on an in-flight collective
                blk_bounds = {}
                _s = 0
                for jc_ in range(JCH):
                    for mi_, (_c, _w) in enumerate(macs(jc_)):
                        blk_bounds[(jc_, mi_)] = (_s, _s + T * (_w // 128))
                        _s += T * (_w // 128)
                pending_zs = []

                def flush_zs(jc_, mi_):
                    cur = blk_bounds[(jc_, mi_)][0]
                    for end_slot, key in list(pending_zs):
                        if cur - end_slot >= 280:
                            zs_load(*key)
                            pending_zs.remove((end_slot, key))

                def transpose_relu(src, rows, jt, col0):
                    """zrT[jt][:, col0:col0+rows] = relu(src[0:rows, jt].T + bias)
                    post-op alternates scalar/vector so neither engine gates
                    the transpose stream."""
                    tp = tlps.tile([128, 128], BF16, name=f"tp_{jt}_{col0}",
                                   tag="tl")
                    nc.tensor.transpose(tp[:, 0:rows],
                                        src[0:rows, jt * 128:(jt + 1) * 128],
                                        ident[0:rows, 0:rows])
                    dst = zrT[jt][:, col0:col0 + rows]
                    if jt % 2 == 0:
                        nc.scalar.activation(dst, tp[:, 0:rows], AF.Relu,
                                             bias=biasall[:, jt:jt + 1])
                    else:
                        nc.vector.tensor_scalar(dst, tp[:, 0:rows],
                                                biasall[:, jt:jt + 1], 0.0,
                                                mybir.AluOpType.add,
                                                mybir.AluOpType.max)

                CH = min(512, DO2)
                NCH = DO2 // CH

                def shadow_gemm_block(b0, bw):
                    """out_a[b0:b0+bw, :] = zrT[:, b0:b0+bw].T @ Wf2 + bf2,
                    batch-major so Wf2 is the moving operand (N=512 streams
                    hide every weight load); bias lands via a K=1 matmul.
                    PSUM comes from the (now idle) GEMM2 pool so blocks
                    pipeline instead of serializing on two tail banks."""
                    for ch in range(NCH):
                        pa = zps.tile([bw, CH], F32, name=f"pa_{b0}_{ch}",
                                      tag="z")
                        for jt in range(NJT):
                            nc.tensor.matmul(
                                pa[:], zrT[jt][:, b0:b0 + bw],
                                wf2_sb[:, jt, ch * CH:(ch + 1) * CH],
                                start=(jt == 0), stop=False,
                                skip_group_check=True)
                        nc.tensor.matmul(pa[:], ones_sb[:, 0:bw],
                                         bf2row[:, ch * CH:(ch + 1) * CH],
                                         start=False, stop=True,
                                         skip_group_check=True)
                        osb = fop.tile([bw, CH], F32, name=f"oa_{b0}_{ch}",
                                       tag="osb")
                        if ch % 2 == 0:
                            nc.vector.tensor_copy(osb[:], pa[:])
                        else:
                            nc.scalar.activation(osb[:], pa[:], AF.Copy)
                        nc.scalar.dma_start(
                            out_a[b0:b0 + bw, ch * CH:(ch + 1) * CH], osb[:])

                def tail_gemm():
                    """out_b = (zrT tail cols).T @ Wf2 + bf2, channel-major,
                    all NCT groups in one PSUM tile and ONE output DMA."""
                    c0 = P1W
                    ob = fop.tile([128, NCT, TLW], F32, name="ob_t", tag="osb")
                    for ct in range(NCT):
                        # per-ct PSUM tiles keep the drain dependencies
                        # precise (a shared tile made every drain wait for
                        # the last group's matmuls)
                        pb = zps.tile([128, TLW], F32, name=f"pb_{ct}",
                                      tag="z")
                        for jt in range(NJT):
                            nc.tensor.matmul(
                                pb[:],
                                wf2_sb[:, jt, ct * 128:(ct + 1) * 128],
                                zrT[jt][:, c0:c0 + TLW],
                                start=(jt == 0), stop=False,
                                skip_group_check=True)
                        # bias via a K=1 matmul (bf2 row outer ones), so the
                        # drains below are pure copies split across engines
                        nc.tensor.matmul(pb[:],
                                         bf2row[:, ct * 128:(ct + 1) * 128],
                                         ones_sb[:, 0:TLW],
                                         start=False, stop=True,
                                         skip_group_check=True)
                        if ct % 2 == 0:
                            nc.vector.tensor_copy(ob[:, ct, :], pb[:])
                        else:
                            nc.scalar.activation(ob[:, ct, :], pb[:],
                                                 AF.Copy)
                    nc.sync.dma_start(
                        out_b.rearrange("(ct p) w -> p ct w", p=128), ob[:])

                for jc in range(JCH):
                    mcs = macs(jc)
                    NMj = len(mcs)
                    for mi, (c0, mw) in enumerate(mcs):
                        nbt = mw // 128
                        flush_zs(jc, mi)
                        # bias = bf1 + sum of the 4 zc rows; placed here (well
                        # after the AllReduce has landed) so the scalar-queue
                        # zcb4 loads never block the h2/zp pipelines
                        if jc == JCH - 1 and mi == min(1, NMj - 1):
                            # tail-only consts, loaded late so the big gpsimd
                            # bursts never overlap the h2-critical window
                            for jt in range(NJT):
                                nc.gpsimd.dma_start(
                                    wf2_sb[:, jt, :],
                                    wf2[jt * 128:(jt + 1) * 128, :])
                            for g in range(8):
                                nc.scalar.dma_start(
                                    zcb4[:, g, :],
                                    zc_out[g:g + 1, :].rearrange(
                                        "o (jt p) -> (o p) jt", p=128))
                            nc.gpsimd.tensor_add(biasall[:], bf1_sb[:],
                                                 zcb4[:, 0, :])
                            for g in range(1, 8):
                                nc.gpsimd.tensor_add(biasall[:], biasall[:],
                                                     zcb4[:, g, :])
                        z_ps = [zps.tile([128, JW], F32, name=f"z_{jc}_{mi}_{bt}",
                                         tag="z")
                                for bt in range(nbt)]
                        for tq in range(T // HQ):
                            ht = h2p.tile([128, HQ, mw], BF16,
                                          name=f"h2_{jc}_{mi}_{tq}", tag="h2")
                            eng = nc.sync if tq % 2 == 0 else nc.scalar
                            eng.dma_start(
                                ht[:],
                                hq_d[tq][:, c0:c0 + mw]
                                .rearrange("(q p) b -> p q b", p=128))
                            for q in range(HQ):
                                t = tq * HQ + q
                                for bt in range(nbt):
                                    nc.tensor.matmul(
                                        z_ps[bt][:],
                                        ht[:, q, bt * 128:(bt + 1) * 128],
                                        w1_tiles[jc][t][:],
                                        start=(t == 0), stop=(t == T - 1),
                                        skip_group_check=True)
                            # deferred last-branch chunk-0 zc packs + u-fold,
                            # well past its (trailing) BN-stats chain and well
                            # before this macro's t-loop reaches those tiles
                            if DEFER and jc == 0 and mi == 0 and tq == 2:
                                zc7_ps = tlps.tile([128, JW], F32,
                                                   name="zc7_ps", tag="tl")
                                nc.vector.memset(zc7_ps[:], 0.0)
                                for ot in range(TPB):
                                    t7 = (KB - 1) * TPB + ot
                                    zc_pack_mm(zc7_ps, 0, t7, T - TPB, T)
                                for ot in range(TPB):
                                    t7 = (KB - 1) * TPB + ot
                                    nc.vector.tensor_scalar_mul(
                                        w1_tiles[0][t7][:], w1_tiles[0][t7][:],
                                        u_all[:, t7:t7 + 1])
                                zc_collect(zc7_ps, zcs7, 0, r0=4)
                        if jc == 0 and mi == 0:
                            for njc in range(1, JCH):
                                w1b_issue(njc, T // 2, T)
                        last_mac = jc == JCH - 1 and mi == NMj - 1
                        for bt in range(nbt):
                            zsb = zsbp.tile([128, JW], BF16,
                                            name=f"zsb_{jc}_{mi}_{bt}", tag="zsb")
                            # alternate drain engines so the stores fan out
                            if bt % 2 == 0:
                                nc.vector.tensor_copy(zsb[:], z_ps[bt][:])
                            else:
                                nc.scalar.activation(zsb[:], z_ps[bt][:],
                                                     AF.Copy)
                            sp_i, row0, nrows = next(
                                p for p in pieces(jc, mi)
                                if p[1] <= bt * 128 < p[1] + p[2])
                            ro = bt * 128 - row0
                            # the last macro's stores split across two queues
                            # to cut the store->RS-trigger latency
                            zeng = nc.sync if (last_mac and bt % 2) else nc.scalar
                            zeng.dma_start(
                                zp[(jc, mi, sp_i)][ro:ro + 128, :], zsb[:])
                            if ro + 128 == nrows:
                                nc.gpsimd.collective_compute(
                                    "ReduceScatter", mybir.AluOpType.add,
                                    replica_groups=[list(range(n_cores))],
                                    ins=[zp[(jc, mi, sp_i)].opt()],
                                    outs=[zs[(jc, mi, sp_i)].opt()])
                                # the very last piece's shard load is deferred
                                # to the tail so it can't block shadow work
                                if not last_mac:
                                    pending_zs.append(
                                        (blk_bounds[(jc, mi)][1],
                                         (jc, mi, sp_i)))
                        # prep of next chunk's zc + scale folding, after this
                        # chunk's SECOND b-macro (by then the w1 chunk-nj DMA
                        # stream on gpsimd has fully landed, so the packed
                        # zc matmuls never stall the PE waiting on tiles)
                        nj = jc + 1
                        if mi == min(1, NMj - 1) and nj < JCH:
                            zcn_ps = tlps.tile([128, JW], F32, name=f"zc_{nj}",
                                               tag="tl")
                            nc.vector.memset(zcn_ps[:], 0.0)
                            for t in range(T):
                                zc_pack_mm(zcn_ps, nj, t, 0, T)
                            for t in range(T):
                                nc.vector.tensor_scalar_mul(
                                    w1_tiles[nj][t][:], w1_tiles[nj][t][:],
                                    u_all[:, t:t + 1])
                            zc_collect(zcn_ps, zcs_t[nj], nj)
                    if jc == 0 or JCH == 1:
                        # all local zc written -> tiny AllReduce, placed on the
                        # gpsimd queue between the two chunks' RS pieces
                        nc.gpsimd.collective_compute(
                            "AllReduce", mybir.AluOpType.add,
                            replica_groups=[list(range(n_cores))],
                            ins=[zc_in.opt()], outs=[zc_out.opt()])

                # ---- tail: everything below the last RS trigger ----
                # Work that only needs already-landed shards runs in the
                # final ReduceScatter's latency shadow: all transposes except
                # the last macro's last-chunk columns, plus the batch-major
                # final GEMM over rows [0, P1W).  Only TLW rows' worth of
                # work remains on the critical path behind the final RS.
                ljc = JCH - 1
                NMl = len(macs(ljc))
                # sync queue, in order: the (possibly still in-flight)
                # last-two shards, then the final shard (blocks on the
                # final RS), then part-2's single output DMA.  Nothing on
                # sync gates the shadow work.
                for end_slot, key in sorted(pending_zs):
                    if key[0] == ljc and key[1] >= NMl - 3:
                        zs_load(*key, eng=nc.sync)
                        pending_zs.remove((end_slot, key))
                # final shard load (sync queue: blocks on the final RS
                # without gating any shadow work)
                zs_load(ljc, NMl - 1, 0, eng=nc.sync)
                for _end, key in sorted(pending_zs):
                    zs_load(*key)
                pending_zs.clear()
                # Shadow schedule: per row tile, transpose all j-tiles then
                # run the covered batch-major final-GEMM block right away —
                # dense matmul work keeps the HAM clock-gate warm through
                # the RS window.  The last tile's post-RS rows transpose as
                # (finite) garbage and are overwritten after the RS.
                b0 = 0
                for zt_, base, nr in zrow:
                    for jt in range(NJT):
                        transpose_relu(zt_, nr, jt, base)
                    avail = min(base + nr, P1W)
                    while b0 + 128 <= avail or (avail == P1W and b0 < avail):
                        bw = min(128, avail - b0)
                        shadow_gemm_block(b0, bw)
                        b0 += bw
                # critical path: last RS -> NJL small transposes -> final
                # GEMM tail.  Transpose from a 32-aligned start partition
                # (engine requirement) and take the last TLW output columns.
                lz, lbase, lnr = zrow[-1]
                tb0 = (lnr - TLW) // 32 * 32
                sp0 = lnr - TLW - tb0
                for k in range(NJL):
                    jt = ljc * NJL + k
                    tpk = tlps.tile([128, lnr - tb0], BF16, name=f"tpk_{k}",
                                    tag="tl")
                    nc.tensor.transpose(
                        tpk[:], lz[tb0:lnr, jt * 128:(jt + 1) * 128],
                        ident[tb0:lnr, tb0:lnr],
                        tile_position=(tb0, 0))
                    dst = zrT[jt][:, P1W:BL]
                    src = tpk[:, sp0:sp0 + TLW]
                    if k % 2 == 0:
                        nc.scalar.activation(dst, src, AF.Relu,
                                             bias=biasall[:, jt:jt + 1])
                    else:
                        nc.vector.tensor_scalar(dst, src,
                                                biasall[:, jt:jt + 1], 0.0,
                                                mybir.AluOpType.add,
                                                mybir.AluOpType.max)
                tail_gemm()

    return nc


def prep_in_maps(cfg, x, W, b, gamma, beta, Wf1, bf1, Wf2, bf2):
    d = _dims(cfg)
    n_cores, DI, DO, DF = cfg["n_cores"], cfg["DI"], cfg["DO"], cfg["DF"]
    KB, T, TPB, NJT, NCT = d["KB"], d["T"], d["TPB"], d["NJT"], d["NCT"]

    xTb = np.ascontiguousarray(np.asarray(x, dtype=np.float32).T.astype(BD))
    wf2b = np.ascontiguousarray(np.asarray(Wf2, dtype=np.float32).astype(BD))
    bf1_rr = np.ascontiguousarray(
        np.asarray(bf1, dtype=np.float32).reshape(NJT, 128).T)
    bf2_rr = np.ascontiguousarray(
        np.asarray(bf2, dtype=np.float32).reshape(NCT, 128).T)
    bf2_fb = np.ascontiguousarray(
        np.asarray(bf2, dtype=np.float32).reshape(1, -1).astype(BD))

    def fold_cols(a_loc):  # [KB, DO] -> [128, T] with col = kb*TPB+ot
        return np.ascontiguousarray(
            np.asarray(a_loc, dtype=np.float32)
            .reshape(KB, TPB, 128).transpose(2, 0, 1).reshape(128, T))

    in_maps = []
    for c in range(n_cores):
        ks = slice(c * KB, (c + 1) * KB)
        wl = np.ascontiguousarray(
            np.asarray(W[ks], dtype=np.float32).reshape(KB * DI, DO).astype(BD))
        w1l = np.ascontiguousarray(
            np.asarray(Wf1[c * KB * DO:(c + 1) * KB * DO], dtype=np.float32)
            .astype(BD))
        in_maps.append({
            "xT": xTb, "wloc": wl, "w1loc": w1l, "wf2": wf2b,
            "b_r": fold_cols(b[ks]), "gamma_r": fold_cols(gamma[ks]),
            "beta_r": fold_cols(beta[ks]),
            "bf1_r": bf1_rr, "bf2_r": bf2_rr, "bf2_f": bf2_fb,
        })
    return in_maps


def assemble_output(cfg, results):
    d = _dims(cfg)
    B, DO2, n_cores = cfg["B"], cfg["DO2"], cfg["n_cores"]
    NBM, BMW, PR, BL = d["NBM"], d["BMW"], d["PR"], d["BL"]
    P1W = d["P1W"]
    blocks = []
    for bm in range(NBM - 1):
        for sp in range(BMW // PR):
            blocks.append((bm * BMW + sp * PR, PR))
    acc = (NBM - 1) * BMW
    for tw in d["TSP"]:
        blocks.append((acc, tw))
        acc += tw
    out = np.empty((B, DO2), dtype=np.float32)
    for c in range(n_cores):
        loc = np.empty((BL, DO2), dtype=np.float32)
        loc[0:P1W] = results[c]["out_a"]
        loc[P1W:BL] = results[c]["out_b"].T
        for c0, nrows in blocks:
            plp = nrows // n_cores
            l0 = c0 // n_cores
            g0 = c0 + c * plp
            out[g0:g0 + plp, :] = loc[l0:l0 + plp, :]
    return out


_COMPILED = None


def _get_compiled():
    global _COMPILED
    if _COMPILED is None:
        nc = build_bass(FULL_CFG)
        nc.compile()
        _COMPILED = nc
    return _COMPILED


def kernel(**inputs):
    cfg = FULL_CFG
    nc = _get_compiled()
    in_maps = prep_in_maps(cfg, **inputs)
    res = run_bass_kernel_spmd(nc, in_maps,
                               core_ids=list(range(cfg["n_cores"])))
    return assemble_output(cfg, res.results)
